# revision 73
# baseline (speedup 1.0000x reference)
"""Trainium2 Bass kernel for nn_Net_76562087018570.

Computation (reference): per-column MinMax scale of a (4096, 8192) f32 matrix,
10 iterations of arr = arr*(1 - (arr - rowmean(arr))) (+0.001 on iter 0),
then inverse transform.  Rows sharded 8 ways (512 rows/core).

v2 design:
- All data f16 in SBUF.  Inflow f32 quarters land in 2 rotating staging
  slots, ACT casts to the persistent f16 tile D while DVE computes the
  per-column min/max combine tree in f16 (2x mode), chasing the input DMA.
- Column min/max: pairwise f16 max/min combines (last step emits f32),
  gpsimd partition_all_reduce for the max side, PE transposes + DVE min
  reduces for the min side, one packed AllReduce(max) on [max | -min].
  A tiny warmup AllReduce + warmup PAR at t=0 pay the collective barrier
  and gpsimd library load during the load phase (saves ~100us).
- Iteration state: ACT slice (A cols) carries s_k = (arr_k - C_k)^2 and
  uses one Square activation per pass (bias=G, scale=-1, fused row-sum).
  DVE slice (V cols) carries y_k = s_k + lambda_k, for which the update
  collapses to ONE scalar_tensor_tensor per pass: y' = (y - 2g)*y at f16
  2x mode, with fused row-sum.  Per-row scalar chains (10 tiny DVE ops
  per group-pass) track h/C/G/lambda.
- Startup ((a-mn)*rinv, f16 2x) and final (mn + safe*(gam - state), f16 2x)
  are two DVE ops each; ACT converts the f16 result to f32 into the dead
  f32 staging slots for DMA out.  Broadcast vectors (mn, rinv, safe) are
  f16.  Two tile groups (2 tiles each) pipeline passes/chains/finals.
"""

import os
import numpy as np

R = 512          # rows per core
N = 8192         # columns
NT = 4           # (128,N) row tiles per core
NQ = 4           # column quarters
QW = N // NQ     # 2048
NCORES = 8
NPASS = 10
A = 5824         # ACT-slice columns (s-state)
V = N - A        # DVE-slice columns (y-state), 2368

_cache = {}
LAST_RESULT = None


def _build():
    import concourse.bacc as bacc
    import concourse.tile as tile
    from concourse import mybir, masks, bass_isa

    f32 = mybir.dt.float32
    f16 = mybir.dt.float16
    Al = mybir.AluOpType
    AF = mybir.ActivationFunctionType
    AX = mybir.AxisListType

    nc = bacc.Bacc(trn_type="TRN2", num_devices=NCORES)
    xs = nc.dram_tensor("xs", [R, N], f32, kind="ExternalInput")
    out = nc.dram_tensor("out", [R, N], f32, kind="ExternalOutput")
    xv = xs.ap().rearrange("(t p) n -> t p n", p=128)
    ov = out.ap().rearrange("(t p) n -> t p n", p=128)

    with tile.TileContext(nc) as tc:
        with tc.tile_pool(name="rot", bufs=1) as rot, \
             tc.tile_pool(name="data", bufs=1) as data, \
             tc.tile_pool(name="mmq", bufs=1) as mmq, \
             tc.tile_pool(name="small", bufs=1) as small, \
             tc.tile_pool(name="psumT", bufs=2, space="PSUM") as psumT, \
             tc.tile_pool(name="dram", bufs=1, space="DRAM") as dram:

            # ---- warmups: collective barrier + ring setup, gpsimd PAR
            # library load, ACT Square table load -- all during the input DMA.
            # The warmup collective must be scheduled FIRST: zero-dep memset
            # source, DMA on the ACT hwdge queue, high scheduler priority.
            with tc.high_priority():
                wz = small.tile([1, 8], f32)
                nc.vector.memset(wz[:], 0.0)
                wc_in = dram.tile([1, 8], f32)
                wc_out = dram.tile([1, 8], f32)
                nc.scalar.dma_start(wc_in[:], wz[:])
                nc.gpsimd.collective_compute(
                    "AllReduce", Al.max,
                    replica_groups=[[0, 1, 2, 3], [4, 5, 6, 7]],
                    ins=[wc_in[:]], outs=[wc_out[:]],
                )
                wq = small.tile([128, 8], f32)
                nc.vector.memset(wq[:], 0.0)
                wsq = small.tile([128, 8], f32)
                nc.scalar.activation(wsq[:], wq[:], AF.Square)

            ident = small.tile([128, 128], f32)
            masks.make_identity(nc, ident[:])
            identh = small.tile([128, 128], f16)
            nc.scalar.copy(identh[:], ident[:])

            # ---- persistent f16 data: D[:, t*N : (t+1)*N] = tile t ----
            D = data.tile([128, NT * N], f16, name="D")
            Dv3 = D[:].rearrange("p (t n) -> p t n", t=NT)

            def dseg(t, lo, hi):
                return D[:, t * N + lo: t * N + hi]

            # ---- phase 1: load quarters into rotating f32 slots, cast to
            # f16 (ACT), combine min/max (DVE f16), PAR (gpsimd) + PE/DVE
            # min reduction, pack collective input ----
            rsl = [rot.tile([128, N], f32, name=f"rs{b}") for b in range(2)]
            cmaxs = [mmq.tile([128, QW], f16, name=f"cmax{b}") for b in range(2)]
            cmins = [mmq.tile([128, QW], f16, name=f"cmin{b}") for b in range(2)]
            rmin = small.tile([128, 64], f32)
            rmax = small.tile([128, 64], f32)
            cc_in = dram.tile([2, N], f16)
            cc_out = dram.tile([2, N], f16, addr_space="Shared")

            def preduce(src, dst, j, op):
                # partition reduce of (128, QW) f16 -> dst[p, j*16+cb]
                # = op over col j*2048 + cb*128 + p, via PE transposes
                for g in range(2):
                    pt = psumT.tile([128, 1024], f16, name="pt", tag="pst")
                    for b8 in range(8):
                        cb = g * 8 + b8
                        nc.tensor.transpose(
                            pt[:, b8 * 128:(b8 + 1) * 128],
                            src[:, cb * 128:(cb + 1) * 128],
                            identh[:])
                    nc.vector.tensor_reduce(
                        out=dst[:, j * 16 + g * 8:j * 16 + g * 8 + 8],
                        in_=pt[:].rearrange("p (c x) -> p c x", c=8),
                        axis=AX.X, op=op)

            for j in range(NQ):
                b = j % 2
                qlo = j * QW
                cmax2, cmin2 = cmaxs[b], cmins[b]
                for t in range(NT):
                    nc.sync.dma_start(rsl[b][:, t * QW:(t + 1) * QW],
                                      xv[t][:, qlo:qlo + QW])
                    # cast per tile as its quarter-slice arrives
                    nc.scalar.copy(dseg(t, qlo, qlo + QW),
                                   rsl[b][:, t * QW:(t + 1) * QW])
                # all-f16 combine trees, in place
                nc.vector.tensor_tensor(cmax2[:], dseg(0, qlo, qlo + QW),
                                        dseg(1, qlo, qlo + QW), op=Al.max)
                nc.vector.tensor_tensor(cmax2[:], cmax2[:],
                                        dseg(2, qlo, qlo + QW), op=Al.max)
                nc.vector.tensor_tensor(cmax2[:], cmax2[:],
                                        dseg(3, qlo, qlo + QW), op=Al.max)
                nc.vector.tensor_tensor(cmin2[:], dseg(0, qlo, qlo + QW),
                                        dseg(1, qlo, qlo + QW), op=Al.min)
                nc.vector.tensor_tensor(cmin2[:], cmin2[:],
                                        dseg(2, qlo, qlo + QW), op=Al.min)
                nc.vector.tensor_tensor(cmin2[:], cmin2[:],
                                        dseg(3, qlo, qlo + QW), op=Al.min)
                preduce(cmax2, rmax, j, Al.max)
                preduce(cmin2, rmin, j, Al.min)

            # pack [max | -min] partition-major as f16
            rmax16 = small.tile([128, 64], f16)
            nc.vector.tensor_scalar(out=rmax16[:], in0=rmax[:], scalar1=1.0,
                                    scalar2=None, op0=Al.mult)
            nrmin16 = small.tile([128, 64], f16)
            nc.vector.tensor_scalar(out=nrmin16[:], in0=rmin[:], scalar1=-1.0,
                                    scalar2=None, op0=Al.mult)
            nc.sync.dma_start(
                cc_in[0:1, :].rearrange("o (p f) -> (o p) f", p=128),
                rmax16[:])
            nc.sync.dma_start(
                cc_in[1:2, :].rearrange("o (p f) -> (o p) f", p=128),
                nrmin16[:])

            # ---- AllReduce(max) on [gmax | -min] ----
            nc.gpsimd.collective_compute(
                "AllReduce", Al.max,
                replica_groups=[list(range(NCORES))],
                ins=[cc_in[:]], outs=[cc_out[:]],
            )

            # ---- post-collective scalar math in partition-major (128,64) ----
            gmaxP = small.tile([128, 64], f16)
            nc.sync.dma_start(
                gmaxP[:],
                cc_out[0:1, :].rearrange("o (p f) -> (o p) f", p=128))
            nminP = small.tile([128, 64], f16)
            nc.sync.dma_start(
                nminP[:],
                cc_out[1:2, :].rearrange("o (p f) -> (o p) f", p=128))

            # fast lane: min path first so the mnb broadcast (startup's
            # first dependency) leaves as early as possible
            minP = small.tile([128, 64], f32)
            nc.vector.tensor_scalar(out=minP[:], in0=nminP[:], scalar1=-1.0,
                                    scalar2=None, op0=Al.mult)
            ta = psumT.tile([64, 128], f32, name="ta", tag="pst")
            nc.tensor.transpose(ta[:], minP[:], ident[:])
            tas = small.tile([64, 128], f16)
            nc.scalar.copy(tas[:], ta[:])
            mn_d = dram.tile([1, N], f16)
            nc.sync.dma_start(
                mn_d[:].rearrange("o (f p) -> (o f) p", p=128), tas[:])
            mnb = data.tile([128, N], f16, name="mnb")
            H = N // 2
            nc.sync.dma_start(mnb[:, 0:H],
                              mn_d[0:1, 0:H].to_broadcast((128, H)))
            nc.scalar.dma_start(mnb[:, H:N],
                                mn_d[0:1, H:N].to_broadcast((128, H)))

            # range path
            rng = small.tile([128, 64], f32)
            nc.vector.tensor_tensor(rng[:], gmaxP[:], nminP[:], op=Al.add)
            eq0 = small.tile([128, 64], f32)
            nc.vector.tensor_scalar(out=eq0[:], in0=rng[:], scalar1=0.0,
                                    scalar2=None, op0=Al.is_equal)
            safe = small.tile([128, 64], f32)
            nc.vector.tensor_tensor(safe[:], rng[:], eq0[:], op=Al.add)
            rinv = small.tile([128, 64], f32)
            nc.vector.reciprocal(rinv[:], safe[:])
            tr = psumT.tile([64, 128], f32, name="tr", tag="pst")
            nc.tensor.transpose(tr[:], rinv[:], ident[:])
            trs = small.tile([64, 128], f16)
            nc.scalar.copy(trs[:], tr[:])
            rinv_d = dram.tile([1, N], f16)
            nc.sync.dma_start(
                rinv_d[:].rearrange("o (f p) -> (o f) p", p=128), trs[:])
            rb = data.tile([128, N], f16, name="rb")
            nc.sync.dma_start(rb[:, 0:H],
                              rinv_d[0:1, 0:H].to_broadcast((128, H)))
            nc.scalar.dma_start(rb[:, H:N],
                                rinv_d[0:1, H:N].to_broadcast((128, H)))

            # safe path (needed only at the finals)
            tb = psumT.tile([64, 128], f32, name="tb", tag="pst")
            nc.tensor.transpose(tb[:], safe[:], ident[:])
            tbs = small.tile([64, 128], f16)
            nc.scalar.copy(tbs[:], tb[:])
            safe_d = dram.tile([1, N], f16)
            nc.sync.dma_start(
                safe_d[:].rearrange("o (f p) -> (o f) p", p=128), tbs[:])

            # ---- startup: arr0 = (a' - mn)*rinv in place, f16 2x ----
            sarr = [small.tile([128, 1], f32, name=f"sarr{t}")
                    for t in range(NT)]

            def startup(t):
                # halved op1 for every tile: the second half's drain expires
                # while the first runs, so op2 (whose accumulate feeds the
                # tile's first-pass bias) isn't deferred behind other tiles
                Dt = dseg(t, 0, N)
                Hh = N // 2
                nc.vector.tensor_tensor(dseg(t, 0, Hh), dseg(t, 0, Hh),
                                        mnb[:, 0:Hh], op=Al.subtract)
                nc.vector.tensor_tensor(dseg(t, Hh, N), dseg(t, Hh, N),
                                        mnb[:, Hh:N], op=Al.subtract)
                nc.vector.scalar_tensor_tensor(
                    out=Dt, in0=Dt, scalar=0.0, in1=rb[:],
                    op0=Al.bypass, op1=Al.mult,
                    accum_out=sarr[t][:])

            def bc_safeb():
                sb = data.tile([128, N], f16, name="rb")
                nc.scalar.dma_start(sb[:],
                                    safe_d[0:1, :].to_broadcast((128, N)))
                return sb

            # ---- per-group (2 tiles) scalar chains, (128,2) f32 tiles ----
            G = [dict() for _ in range(2)]

            _ntc = [0]

            def nt_(shape=(128, 2)):
                _ntc[0] += 1
                return small.tile(list(shape), f32, name=f"ch{_ntc[0]}")

            def ginitA(t):
                # per-tile C0/g2, computed ON ACT (Copy supports float
                # bias + scale) so ACT feeds itself its first bias without
                # waiting behind big ops in the DVE queue
                Ct = nt_((128, 1))
                nc.scalar.activation(Ct[:], sarr[t][:], AF.Copy,
                                     bias=0.5, scale=0.5 / N)
                g2t = nt_((128, 1))
                nc.scalar.activation(g2t[:], Ct[:], AF.Copy,
                                     bias=0.0, scale=2.0)
                return Ct, g2t

            def gprep(g):
                st = G[g]
                st["accA"], st["accD"] = nt_(), nt_()

            def gpass0_tile(g, i, t, Ct, g2t):
                st = G[g]
                nc.scalar.activation(
                    dseg(t, 0, A), dseg(t, 0, A), AF.Square,
                    bias=Ct[:], scale=-1.0,
                    accum_out=st["accA"][:, i:i + 1])
                DtV = dseg(t, A, N)
                nc.vector.scalar_tensor_tensor(
                    out=DtV, in0=DtV, scalar=g2t[:], in1=DtV,
                    op0=Al.subtract, op1=Al.mult,
                    accum_out=st["accD"][:, i:i + 1])

            def ginitB(g, C0, C1):
                st = G[g]
                Cp = nt_()
                nc.vector.tensor_copy(Cp[:, 0:1], C0[:])
                nc.vector.tensor_copy(Cp[:, 1:2], C1[:])
                qr = nt_()
                nc.vector.tensor_tensor(qr[:], Cp[:], Cp[:], op=Al.mult)
                q = nt_()
                nc.vector.tensor_scalar(out=q[:], in0=qr[:], scalar1=0.001,
                                        scalar2=None, op0=Al.add)
                Lm = nt_()
                nc.vector.scalar_tensor_tensor(
                    out=Lm[:], in0=Cp[:], scalar=-2.0, in1=Cp[:],
                    op0=Al.mult, op1=Al.mult)
                mu = nt_()
                nc.vector.tensor_scalar(out=mu[:], in0=Lm[:], scalar1=V / 2.0,
                                        scalar2=None, op0=Al.mult)
                st.update(q=q, Lm=Lm, mu=mu)

            def gpass(g, k, last=False):
                st = G[g]
                accA = nt_() if not last else None
                accD = nt_() if not last else None
                st["accA"], st["accD"] = accA, accD
                for i, t in enumerate((2 * g, 2 * g + 1)):
                    nc.scalar.activation(
                        dseg(t, 0, A), dseg(t, 0, A), AF.Square,
                        bias=st["bias"][:, i:i + 1], scale=-1.0,
                        accum_out=(None if last else accA[:, i:i + 1]))
                    DtV = dseg(t, A, N)
                    nc.vector.scalar_tensor_tensor(
                        out=DtV, in0=DtV, scalar=st["g2"][:, i:i + 1],
                        in1=DtV, op0=Al.subtract, op1=Al.mult,
                        accum_out=(None if last else accD[:, i:i + 1]))

            def gchain(g):
                st = G[g]
                u = nt_()
                nc.vector.tensor_tensor(u[:], st["accA"][:], st["accD"][:],
                                        op=Al.add)
                S = nt_()
                nc.vector.tensor_tensor(S[:], u[:], st["mu"][:],
                                        op=Al.subtract)
                t1 = nt_()
                nc.vector.tensor_scalar(out=t1[:], in0=S[:],
                                        scalar1=-1.0 / N, scalar2=None,
                                        op0=Al.mult)
                h = nt_()
                nc.vector.tensor_tensor(h[:], t1[:], st["q"][:], op=Al.add)
                C2 = nt_()
                nc.vector.tensor_scalar(out=C2[:], in0=h[:], scalar1=0.5,
                                        scalar2=0.5, op0=Al.mult, op1=Al.add)
                Gb = nt_()
                nc.vector.tensor_tensor(Gb[:], st["q"][:], C2[:],
                                        op=Al.subtract)
                q2 = nt_()
                nc.vector.tensor_tensor(q2[:], C2[:], C2[:], op=Al.mult)
                g2 = nt_()
                nc.vector.scalar_tensor_tensor(
                    out=g2[:], in0=Gb[:], scalar=2.0, in1=st["Lm"][:],
                    op0=Al.mult, op1=Al.add)
                Lm2 = nt_()
                nc.vector.scalar_tensor_tensor(
                    out=Lm2[:], in0=g2[:], scalar=-0.5, in1=g2[:],
                    op0=Al.mult, op1=Al.mult)
                mu2 = nt_()
                nc.vector.tensor_scalar(out=mu2[:], in0=Lm2[:],
                                        scalar1=V / 2.0, scalar2=None,
                                        op0=Al.mult)
                st.update(q=q2, Lm=Lm2, mu=mu2, bias=Gb, g2=g2)

            def gfinal_scalars(g):
                st = G[g]
                gamV = nt_()
                nc.vector.scalar_tensor_tensor(
                    out=gamV[:], in0=st["Lm"][:], scalar=0.5, in1=st["q"][:],
                    op0=Al.mult, op1=Al.add)
                st["gamV"] = gamV      # = q9 + lambda9 for the y slice
                st["gamA"] = st["q"]   # = q9 = C9^2 for the s slice

            def final(t, safeb):
                # fully half-pipelined: DVE ops, convert, and store per
                # half so the ACT convert and output DMA start early
                g, i = t // 2, t % 2
                st = G[g]
                stag = rot.tile([128, N], f32, name=f"rs{t % 2}")
                for h in range(2):
                    lo, hi = h * (N // 2), (h + 1) * (N // 2)
                    # w = state - gam (4x tensor_scalar, per-row AP scalar);
                    # the A/V boundary (A > N/2) splits the subtracts
                    if lo < A:
                        sa = dseg(t, lo, min(hi, A))
                        nc.vector.tensor_scalar(
                            out=sa, in0=sa, scalar1=st["gamA"][:, i:i + 1],
                            scalar2=None, op0=Al.subtract)
                    if hi > A:
                        sv = dseg(t, max(lo, A), hi)
                        nc.vector.tensor_scalar(
                            out=sv, in0=sv, scalar1=st["gamV"][:, i:i + 1],
                            scalar2=None, op0=Al.subtract)
                    Dh = dseg(t, lo, hi)
                    nc.vector.tensor_tensor(Dh, Dh, safeb[:, lo:hi],
                                            op=Al.mult)
                    nc.vector.tensor_tensor(Dh, mnb[:, lo:hi], Dh,
                                            op=Al.subtract)
                    nc.scalar.copy(stag[:, lo:hi], Dh)
                    nc.sync.dma_start(ov[t][:, lo:hi], stag[:, lo:hi])

            # ---- schedule: G0 = tiles 0,1 runs ~2 passes ahead of G1;
            # first pass per tile so ACT starts right after startup(0).
            startup(0)
            gi0 = ginitA(0)
            gprep(0)
            gpass0_tile(0, 0, 0, *gi0)
            startup(1)
            gi1 = ginitA(1)
            gpass0_tile(0, 1, 1, *gi1)
            ginitB(0, gi0[0], gi1[0])
            gchain(0)
            gpass(0, 1)
            gchain(0)
            gpass(0, 2)
            startup(2)
            gi2 = ginitA(2)
            gchain(0)
            gpass(0, 3)
            startup(3)
            gi3 = ginitA(3)
            safeb = bc_safeb()
            gchain(0)
            gpass(0, 4)
            gprep(1)
            gpass0_tile(1, 0, 2, *gi2)
            gpass0_tile(1, 1, 3, *gi3)
            ginitB(1, gi2[0], gi3[0])
            for k in range(5, NPASS):
                gchain(0)
                gpass(0, k, last=(k == NPASS - 1))
                gchain(1)
                gpass(1, k - 4)
            gfinal_scalars(0)
            final(0, safeb)
            gchain(1)
            gpass(1, 6)
            final(1, safeb)
            gchain(1)
            gpass(1, 7)
            gchain(1)
            gpass(1, 8)
            gchain(1)
            gfinal_scalars(1)
            # last pass per tile so each final starts as soon as its own
            # tile's pass 9 is done
            st1 = G[1]
            st1["accA"], st1["accD"] = None, None
            for i, t in enumerate((2, 3)):
                nc.scalar.activation(
                    dseg(t, 0, A), dseg(t, 0, A), AF.Square,
                    bias=st1["bias"][:, i:i + 1], scale=-1.0)
                DtV = dseg(t, A, N)
                nc.vector.scalar_tensor_tensor(
                    out=DtV, in0=DtV, scalar=st1["g2"][:, i:i + 1],
                    in1=DtV, op0=Al.subtract, op1=Al.mult)
                final(t, safeb)

    if not nc.is_finalized():
        nc.finalize()
    return nc


def _get_nc():
    if "nc" not in _cache:
        _cache["nc"] = _build()
    return _cache["nc"]


def kernel(x):
    global LAST_RESULT
    from concourse.bass_utils import run_bass_kernel_spmd

    x = np.ascontiguousarray(np.asarray(x), dtype=np.float32)
    a = x.reshape(NCORES * R, N)
    nc = _get_nc()
    in_maps = [{"xs": np.ascontiguousarray(a[c * R:(c + 1) * R])}
               for c in range(NCORES)]
    res = run_bass_kernel_spmd(
        nc, in_maps, core_ids=list(range(NCORES)),
        trace=bool(int(os.environ.get("KBENCH_TRACE", "0"))),
    )
    LAST_RESULT = res
    full = np.concatenate([res.results[c]["out"] for c in range(NCORES)], axis=0)
    return full.reshape(1, NCORES * R, N).astype(np.float32)


# revision 74
# speedup vs baseline: 1.0180x; 1.0180x over previous
"""Trainium2 Bass kernel for nn_Net_76562087018570.

Computation (reference): per-column MinMax scale of a (4096, 8192) f32 matrix,
10 iterations of arr = arr*(1 - (arr - rowmean(arr))) (+0.001 on iter 0),
then inverse transform.  Rows sharded 8 ways (512 rows/core).

v2 design:
- All data f16 in SBUF.  Inflow f32 quarters land in 2 rotating staging
  slots, ACT casts to the persistent f16 tile D while DVE computes the
  per-column min/max combine tree in f16 (2x mode), chasing the input DMA.
- Column min/max: pairwise f16 max/min combines (last step emits f32),
  gpsimd partition_all_reduce for the max side, PE transposes + DVE min
  reduces for the min side, one packed AllReduce(max) on [max | -min].
  A tiny warmup AllReduce + warmup PAR at t=0 pay the collective barrier
  and gpsimd library load during the load phase (saves ~100us).
- Iteration state: ACT slice (A cols) carries s_k = (arr_k - C_k)^2 and
  uses one Square activation per pass (bias=G, scale=-1, fused row-sum).
  DVE slice (V cols) carries y_k = s_k + lambda_k, for which the update
  collapses to ONE scalar_tensor_tensor per pass: y' = (y - 2g)*y at f16
  2x mode, with fused row-sum.  Per-row scalar chains (10 tiny DVE ops
  per group-pass) track h/C/G/lambda.
- Startup ((a-mn)*rinv, f16 2x) and final (mn + safe*(gam - state), f16 2x)
  are two DVE ops each; ACT converts the f16 result to f32 into the dead
  f32 staging slots for DMA out.  Broadcast vectors (mn, rinv, safe) are
  f16.  Two tile groups (2 tiles each) pipeline passes/chains/finals.
"""

import os
import numpy as np

R = 512          # rows per core
N = 8192         # columns
NT = 4           # (128,N) row tiles per core
NQ = 4           # column quarters
QW = N // NQ     # 2048
NCORES = 8
NPASS = 10
A = 5824         # ACT-slice columns (s-state)
V = N - A        # DVE-slice columns (y-state), 2368

_cache = {}
LAST_RESULT = None


def _build():
    import concourse.bacc as bacc
    import concourse.tile as tile
    from concourse import mybir, masks, bass_isa

    f32 = mybir.dt.float32
    f16 = mybir.dt.float16
    Al = mybir.AluOpType
    AF = mybir.ActivationFunctionType
    AX = mybir.AxisListType

    nc = bacc.Bacc(trn_type="TRN2", num_devices=NCORES)
    xs = nc.dram_tensor("xs", [R, N], f32, kind="ExternalInput")
    out = nc.dram_tensor("out", [R, N], f32, kind="ExternalOutput")
    xv = xs.ap().rearrange("(t p) n -> t p n", p=128)
    ov = out.ap().rearrange("(t p) n -> t p n", p=128)

    with tile.TileContext(nc) as tc:
        with tc.tile_pool(name="rot", bufs=1) as rot, \
             tc.tile_pool(name="data", bufs=1) as data, \
             tc.tile_pool(name="mmq", bufs=1) as mmq, \
             tc.tile_pool(name="small", bufs=1) as small, \
             tc.tile_pool(name="psumT", bufs=2, space="PSUM") as psumT, \
             tc.tile_pool(name="dram", bufs=1, space="DRAM") as dram:

            # ---- warmups: collective barrier + ring setup, gpsimd PAR
            # library load, ACT Square table load -- all during the input DMA.
            # The warmup collective must be scheduled FIRST: zero-dep memset
            # source, DMA on the ACT hwdge queue, high scheduler priority.
            with tc.high_priority():
                wz = small.tile([1, 8], f32)
                nc.vector.memset(wz[:], 0.0)
                wc_in = dram.tile([1, 8], f32)
                wc_out = dram.tile([1, 8], f32)
                nc.scalar.dma_start(wc_in[:], wz[:])
                nc.gpsimd.collective_compute(
                    "AllReduce", Al.max,
                    replica_groups=[[0, 1, 2, 3], [4, 5, 6, 7]],
                    ins=[wc_in[:]], outs=[wc_out[:]],
                )
                wq = small.tile([128, 8], f32)
                nc.vector.memset(wq[:], 0.0)
                wsq = small.tile([128, 8], f32)
                nc.scalar.activation(wsq[:], wq[:], AF.Square)

            ident = small.tile([128, 128], f32)
            masks.make_identity(nc, ident[:])
            identh = small.tile([128, 128], f16)
            nc.scalar.copy(identh[:], ident[:])

            # ---- persistent f16 data: D[:, t*N : (t+1)*N] = tile t ----
            D = data.tile([128, NT * N], f16, name="D")
            Dv3 = D[:].rearrange("p (t n) -> p t n", t=NT)

            def dseg(t, lo, hi):
                return D[:, t * N + lo: t * N + hi]

            # ---- phase 1: load quarters into rotating f32 slots, cast to
            # f16 (ACT), combine min/max (DVE f16), PAR (gpsimd) + PE/DVE
            # min reduction, pack collective input ----
            rsl = [rot.tile([128, N], f32, name=f"rs{b}") for b in range(2)]
            cmaxs = [mmq.tile([128, QW], f16, name=f"cmax{b}") for b in range(2)]
            cmins = [mmq.tile([128, QW], f16, name=f"cmin{b}") for b in range(2)]
            rmin = small.tile([128, 64], f32)
            rmax = small.tile([128, 64], f32)
            cc_in = dram.tile([2, N], f16)
            cc_out = dram.tile([2, N], f16, addr_space="Shared")

            def preduce(src, dst, j, op):
                # partition reduce of (128, QW) f16 -> dst[p, j*16+cb]
                # = op over col j*2048 + cb*128 + p, via PE transposes
                for g in range(2):
                    pt = psumT.tile([128, 1024], f16, name="pt", tag="pst")
                    for b8 in range(8):
                        cb = g * 8 + b8
                        nc.tensor.transpose(
                            pt[:, b8 * 128:(b8 + 1) * 128],
                            src[:, cb * 128:(cb + 1) * 128],
                            identh[:])
                    nc.vector.tensor_reduce(
                        out=dst[:, j * 16 + g * 8:j * 16 + g * 8 + 8],
                        in_=pt[:].rearrange("p (c x) -> p c x", c=8),
                        axis=AX.X, op=op)

            for j in range(NQ):
                b = j % 2
                qlo = j * QW
                cmax2, cmin2 = cmaxs[b], cmins[b]
                for t in range(NT):
                    nc.sync.dma_start(rsl[b][:, t * QW:(t + 1) * QW],
                                      xv[t][:, qlo:qlo + QW])
                    # cast per tile as its quarter-slice arrives
                    nc.scalar.copy(dseg(t, qlo, qlo + QW),
                                   rsl[b][:, t * QW:(t + 1) * QW])
                # all-f16 combine trees, in place
                nc.vector.tensor_tensor(cmax2[:], dseg(0, qlo, qlo + QW),
                                        dseg(1, qlo, qlo + QW), op=Al.max)
                nc.vector.tensor_tensor(cmax2[:], cmax2[:],
                                        dseg(2, qlo, qlo + QW), op=Al.max)
                nc.vector.tensor_tensor(cmax2[:], cmax2[:],
                                        dseg(3, qlo, qlo + QW), op=Al.max)
                nc.vector.tensor_tensor(cmin2[:], dseg(0, qlo, qlo + QW),
                                        dseg(1, qlo, qlo + QW), op=Al.min)
                nc.vector.tensor_tensor(cmin2[:], cmin2[:],
                                        dseg(2, qlo, qlo + QW), op=Al.min)
                nc.vector.tensor_tensor(cmin2[:], cmin2[:],
                                        dseg(3, qlo, qlo + QW), op=Al.min)
                preduce(cmax2, rmax, j, Al.max)
                preduce(cmin2, rmin, j, Al.min)

            # pack [max | -min] partition-major as f16
            rmax16 = small.tile([128, 64], f16)
            nc.vector.tensor_scalar(out=rmax16[:], in0=rmax[:], scalar1=1.0,
                                    scalar2=None, op0=Al.mult)
            nrmin16 = small.tile([128, 64], f16)
            nc.vector.tensor_scalar(out=nrmin16[:], in0=rmin[:], scalar1=-1.0,
                                    scalar2=None, op0=Al.mult)
            nc.sync.dma_start(
                cc_in[0:1, :].rearrange("o (p f) -> (o p) f", p=128),
                rmax16[:])
            nc.sync.dma_start(
                cc_in[1:2, :].rearrange("o (p f) -> (o p) f", p=128),
                nrmin16[:])

            # ---- AllReduce(max) on [gmax | -min] ----
            nc.gpsimd.collective_compute(
                "AllReduce", Al.max,
                replica_groups=[list(range(NCORES))],
                ins=[cc_in[:]], outs=[cc_out[:]],
            )

            # ---- post-collective scalar math in partition-major (128,64) ----
            gmaxP = small.tile([128, 64], f16)
            nc.sync.dma_start(
                gmaxP[:],
                cc_out[0:1, :].rearrange("o (p f) -> (o p) f", p=128))
            nminP = small.tile([128, 64], f16)
            nc.sync.dma_start(
                nminP[:],
                cc_out[1:2, :].rearrange("o (p f) -> (o p) f", p=128))

            # fast lane: min path first so the mnb broadcast (startup's
            # first dependency) leaves as early as possible
            minP = small.tile([128, 64], f32)
            nc.vector.tensor_scalar(out=minP[:], in0=nminP[:], scalar1=-1.0,
                                    scalar2=None, op0=Al.mult)
            ta = psumT.tile([64, 128], f32, name="ta", tag="pst")
            nc.tensor.transpose(ta[:], minP[:], ident[:])
            tas = small.tile([64, 128], f16)
            nc.scalar.copy(tas[:], ta[:])
            mn_d = dram.tile([1, N], f16)
            nc.sync.dma_start(
                mn_d[:].rearrange("o (f p) -> (o f) p", p=128), tas[:])
            mnb = data.tile([128, N], f16, name="mnb")
            H = N // 2
            nc.sync.dma_start(mnb[:, 0:H],
                              mn_d[0:1, 0:H].to_broadcast((128, H)))
            nc.scalar.dma_start(mnb[:, H:N],
                                mn_d[0:1, H:N].to_broadcast((128, H)))

            # range path
            rng = small.tile([128, 64], f32)
            nc.vector.tensor_tensor(rng[:], gmaxP[:], nminP[:], op=Al.add)
            eq0 = small.tile([128, 64], f32)
            nc.vector.tensor_scalar(out=eq0[:], in0=rng[:], scalar1=0.0,
                                    scalar2=None, op0=Al.is_equal)
            safe = small.tile([128, 64], f32)
            nc.vector.tensor_tensor(safe[:], rng[:], eq0[:], op=Al.add)
            rinv = small.tile([128, 64], f32)
            nc.vector.reciprocal(rinv[:], safe[:])
            tr = psumT.tile([64, 128], f32, name="tr", tag="pst")
            nc.tensor.transpose(tr[:], rinv[:], ident[:])
            trs = small.tile([64, 128], f16)
            nc.scalar.copy(trs[:], tr[:])
            rinv_d = dram.tile([1, N], f16)
            nc.sync.dma_start(
                rinv_d[:].rearrange("o (f p) -> (o f) p", p=128), trs[:])
            rb = data.tile([128, N], f16, name="rb")
            nc.sync.dma_start(rb[:, 0:H],
                              rinv_d[0:1, 0:H].to_broadcast((128, H)))
            nc.scalar.dma_start(rb[:, H:N],
                                rinv_d[0:1, H:N].to_broadcast((128, H)))

            # safe path (needed only at the finals)
            tb = psumT.tile([64, 128], f32, name="tb", tag="pst")
            nc.tensor.transpose(tb[:], safe[:], ident[:])
            tbs = small.tile([64, 128], f16)
            nc.scalar.copy(tbs[:], tb[:])
            safe_d = dram.tile([1, N], f16)
            nc.sync.dma_start(
                safe_d[:].rearrange("o (f p) -> (o f) p", p=128), tbs[:])

            # ---- startup: arr0 = (a' - mn)*rinv in place, f16 2x ----
            sarr = [small.tile([128, 1], f32, name=f"sarr{t}")
                    for t in range(NT)]

            def startup(t):
                # halved op1 for every tile: the second half's drain expires
                # while the first runs, so op2 (whose accumulate feeds the
                # tile's first-pass bias) isn't deferred behind other tiles
                Dt = dseg(t, 0, N)
                Hh = N // 2
                nc.vector.tensor_tensor(dseg(t, 0, Hh), dseg(t, 0, Hh),
                                        mnb[:, 0:Hh], op=Al.subtract)
                nc.vector.tensor_tensor(dseg(t, Hh, N), dseg(t, Hh, N),
                                        mnb[:, Hh:N], op=Al.subtract)
                nc.vector.scalar_tensor_tensor(
                    out=Dt, in0=Dt, scalar=0.0, in1=rb[:],
                    op0=Al.bypass, op1=Al.mult,
                    accum_out=sarr[t][:])

            def bc_safeb():
                sb = data.tile([128, N], f16, name="rb")
                nc.scalar.dma_start(sb[:],
                                    safe_d[0:1, :].to_broadcast((128, N)))
                return sb

            # ---- per-group (2 tiles) scalar chains, (128,2) f32 tiles ----
            G = [dict() for _ in range(2)]

            _ntc = [0]

            def nt_(shape=(128, 2)):
                _ntc[0] += 1
                return small.tile(list(shape), f32, name=f"ch{_ntc[0]}")

            def ginitA(t):
                # per-tile C0/g2, computed ON ACT (Copy supports float
                # bias + scale) so ACT feeds itself its first bias without
                # waiting behind big ops in the DVE queue
                Ct = nt_((128, 1))
                nc.scalar.activation(Ct[:], sarr[t][:], AF.Copy,
                                     bias=0.5, scale=0.5 / N)
                g2t = nt_((128, 1))
                nc.scalar.activation(g2t[:], Ct[:], AF.Copy,
                                     bias=0.0, scale=2.0)
                return Ct, g2t

            def gprep(g):
                st = G[g]
                st["accA"], st["accD"] = nt_(), nt_()

            def gpass0_tile(g, i, t, Ct, g2t):
                st = G[g]
                nc.scalar.activation(
                    dseg(t, 0, A), dseg(t, 0, A), AF.Square,
                    bias=Ct[:], scale=-1.0,
                    accum_out=st["accA"][:, i:i + 1])
                DtV = dseg(t, A, N)
                nc.vector.scalar_tensor_tensor(
                    out=DtV, in0=DtV, scalar=g2t[:], in1=DtV,
                    op0=Al.subtract, op1=Al.mult,
                    accum_out=st["accD"][:, i:i + 1])

            def ginitB(g, C0, C1):
                st = G[g]
                Cp = nt_()
                nc.vector.tensor_copy(Cp[:, 0:1], C0[:])
                nc.vector.tensor_copy(Cp[:, 1:2], C1[:])
                qr = nt_()
                nc.vector.tensor_tensor(qr[:], Cp[:], Cp[:], op=Al.mult)
                q = nt_()
                nc.vector.tensor_scalar(out=q[:], in0=qr[:], scalar1=0.001,
                                        scalar2=None, op0=Al.add)
                Lm = nt_()
                nc.vector.scalar_tensor_tensor(
                    out=Lm[:], in0=Cp[:], scalar=-2.0, in1=Cp[:],
                    op0=Al.mult, op1=Al.mult)
                qm = nt_()
                nc.vector.scalar_tensor_tensor(
                    out=qm[:], in0=Lm[:], scalar=V / (2.0 * N), in1=q[:],
                    op0=Al.mult, op1=Al.add)
                st.update(q=q, Lm=Lm, qm=qm)

            def gpass(g, k, last=False):
                st = G[g]
                accA = nt_() if not last else None
                accD = nt_() if not last else None
                st["accA"], st["accD"] = accA, accD
                for i, t in enumerate((2 * g, 2 * g + 1)):
                    nc.scalar.activation(
                        dseg(t, 0, A), dseg(t, 0, A), AF.Square,
                        bias=st["bias"][:, i:i + 1], scale=-1.0,
                        accum_out=(None if last else accA[:, i:i + 1]))
                    DtV = dseg(t, A, N)
                    nc.vector.scalar_tensor_tensor(
                        out=DtV, in0=DtV, scalar=st["g2"][:, i:i + 1],
                        in1=DtV, op0=Al.subtract, op1=Al.mult,
                        accum_out=(None if last else accD[:, i:i + 1]))

            def gchain(g):
                st = G[g]
                # h_{k+1} = qm_k - (accA+accD)/N, with qm = q + V*lambda/N
                u = nt_()
                nc.vector.tensor_tensor(u[:], st["accA"][:], st["accD"][:],
                                        op=Al.add)
                h = nt_()
                nc.vector.scalar_tensor_tensor(
                    out=h[:], in0=u[:], scalar=-1.0 / N, in1=st["qm"][:],
                    op0=Al.mult, op1=Al.add)
                C2 = nt_()
                nc.vector.tensor_scalar(out=C2[:], in0=h[:], scalar1=0.5,
                                        scalar2=0.5, op0=Al.mult, op1=Al.add)
                Gb = nt_()
                nc.vector.tensor_tensor(Gb[:], st["q"][:], C2[:],
                                        op=Al.subtract)
                q2 = nt_()
                nc.vector.tensor_tensor(q2[:], C2[:], C2[:], op=Al.mult)
                g2 = nt_()
                nc.vector.scalar_tensor_tensor(
                    out=g2[:], in0=Gb[:], scalar=2.0, in1=st["Lm"][:],
                    op0=Al.mult, op1=Al.add)
                Lm2 = nt_()
                nc.vector.scalar_tensor_tensor(
                    out=Lm2[:], in0=g2[:], scalar=-0.5, in1=g2[:],
                    op0=Al.mult, op1=Al.mult)
                qm2 = nt_()
                nc.vector.scalar_tensor_tensor(
                    out=qm2[:], in0=Lm2[:], scalar=V / (2.0 * N), in1=q2[:],
                    op0=Al.mult, op1=Al.add)
                st.update(q=q2, Lm=Lm2, qm=qm2, bias=Gb, g2=g2)

            def gfinal_scalars(g):
                st = G[g]
                gamV = nt_()
                nc.vector.scalar_tensor_tensor(
                    out=gamV[:], in0=st["Lm"][:], scalar=0.5, in1=st["q"][:],
                    op0=Al.mult, op1=Al.add)
                st["gamV"] = gamV      # = q9 + lambda9 for the y slice
                st["gamA"] = st["q"]   # = q9 = C9^2 for the s slice

            def final(t, safeb):
                # fully half-pipelined: DVE ops, convert, and store per
                # half so the ACT convert and output DMA start early
                g, i = t // 2, t % 2
                st = G[g]
                stag = rot.tile([128, N], f32, name=f"rs{t % 2}")
                for h in range(2):
                    lo, hi = h * (N // 2), (h + 1) * (N // 2)
                    # w = state - gam (4x tensor_scalar, per-row AP scalar);
                    # the A/V boundary (A > N/2) splits the subtracts
                    if lo < A:
                        sa = dseg(t, lo, min(hi, A))
                        nc.vector.tensor_scalar(
                            out=sa, in0=sa, scalar1=st["gamA"][:, i:i + 1],
                            scalar2=None, op0=Al.subtract)
                    if hi > A:
                        sv = dseg(t, max(lo, A), hi)
                        nc.vector.tensor_scalar(
                            out=sv, in0=sv, scalar1=st["gamV"][:, i:i + 1],
                            scalar2=None, op0=Al.subtract)
                    Dh = dseg(t, lo, hi)
                    nc.vector.tensor_tensor(Dh, Dh, safeb[:, lo:hi],
                                            op=Al.mult)
                    nc.vector.tensor_tensor(Dh, mnb[:, lo:hi], Dh,
                                            op=Al.subtract)
                    nc.scalar.copy(stag[:, lo:hi], Dh)
                    nc.sync.dma_start(ov[t][:, lo:hi], stag[:, lo:hi])

            # ---- schedule: G0 = tiles 0,1 runs ~2 passes ahead of G1;
            # first pass per tile so ACT starts right after startup(0).
            startup(0)
            gi0 = ginitA(0)
            gprep(0)
            gpass0_tile(0, 0, 0, *gi0)
            startup(1)
            gi1 = ginitA(1)
            gpass0_tile(0, 1, 1, *gi1)
            ginitB(0, gi0[0], gi1[0])
            gchain(0)
            gpass(0, 1)
            gchain(0)
            gpass(0, 2)
            startup(2)
            gi2 = ginitA(2)
            gchain(0)
            gpass(0, 3)
            startup(3)
            gi3 = ginitA(3)
            safeb = bc_safeb()
            gchain(0)
            gpass(0, 4)
            gprep(1)
            gpass0_tile(1, 0, 2, *gi2)
            gpass0_tile(1, 1, 3, *gi3)
            ginitB(1, gi2[0], gi3[0])
            for k in range(5, NPASS):
                gchain(0)
                gpass(0, k, last=(k == NPASS - 1))
                gchain(1)
                gpass(1, k - 4)
            gfinal_scalars(0)
            final(0, safeb)
            gchain(1)
            gpass(1, 6)
            final(1, safeb)
            gchain(1)
            gpass(1, 7)
            gchain(1)
            gpass(1, 8)
            gchain(1)
            gfinal_scalars(1)
            # last pass per tile so each final starts as soon as its own
            # tile's pass 9 is done
            st1 = G[1]
            st1["accA"], st1["accD"] = None, None
            for i, t in enumerate((2, 3)):
                nc.scalar.activation(
                    dseg(t, 0, A), dseg(t, 0, A), AF.Square,
                    bias=st1["bias"][:, i:i + 1], scale=-1.0)
                DtV = dseg(t, A, N)
                nc.vector.scalar_tensor_tensor(
                    out=DtV, in0=DtV, scalar=st1["g2"][:, i:i + 1],
                    in1=DtV, op0=Al.subtract, op1=Al.mult)
                final(t, safeb)

    if not nc.is_finalized():
        nc.finalize()
    return nc


def _get_nc():
    if "nc" not in _cache:
        _cache["nc"] = _build()
    return _cache["nc"]


def kernel(x):
    global LAST_RESULT
    from concourse.bass_utils import run_bass_kernel_spmd

    x = np.ascontiguousarray(np.asarray(x), dtype=np.float32)
    a = x.reshape(NCORES * R, N)
    nc = _get_nc()
    in_maps = [{"xs": np.ascontiguousarray(a[c * R:(c + 1) * R])}
               for c in range(NCORES)]
    res = run_bass_kernel_spmd(
        nc, in_maps, core_ids=list(range(NCORES)),
        trace=bool(int(os.environ.get("KBENCH_TRACE", "0"))),
    )
    LAST_RESULT = res
    full = np.concatenate([res.results[c]["out"] for c in range(NCORES)], axis=0)
    return full.reshape(1, NCORES * R, N).astype(np.float32)
